# revision 6
# baseline (speedup 1.0000x reference)
"""Trainium2 Bass kernel: KroneckerProductFusionClassifier.

Math (see reference):
    p = x[:, :512]; m = x[:, 512:1280]
    fused[b,k] = sum_{p,m} p[b,p] * m[b,m] * W1r[k,p,m] + b1[k]      (103 GFLOP)
    h = relu(LayerNorm(fused) * ln_g + ln_b)
    h = relu(bn1(h @ W2.T + b2)); h = relu(bn2(h @ W3.T + b3))
    logits = h @ W4.T + b4                                            (tiny tail)

Distribution (8 NeuronCores):
  Stage 1: shard the contraction index p (512 -> 64/core). Each core:
    for p_loc, m_chunk:  scaled[m,b] = mT[m,b] * pT[p,b]   (DVE, bf16)
                         PSUM[b_tile,k] += scaled[:,bt].T @ W[p,mc][m,k]
    -> partial fused [256, 512] fp32 per core (summed over its p range).
  Stage 2: shard batch (256 -> 32/core). Sum the 8 partials, +b1, LayerNorm,
    ReLU, then the small MLP (PE-transpose + fp32 matmuls).

W1 is streamed as bf16 (fused rel-err ~3e-3); set KRON_DTYPE=fp32 to run
the stage-1 matmuls in full fp32 (4x PE cycles, 2x DMA) if tighter accuracy
is ever needed.
"""

import os

import numpy as np
import ml_dtypes

import concourse.bass as bass
import concourse.tile as tile
from concourse import bacc, mybir
from concourse.bass import ds
from concourse.bass_utils import run_bass_kernel_spmd
from concourse.masks import make_identity

B = 256
P_DIM, M_DIM = 512, 768
K = 512          # PROJ
HID, H2, NCLS = 256, 128, 2
EPS = 1e-5
NCORES = 8
PLOC = P_DIM // NCORES      # 64 p-values per core
MCH = M_DIM // 128          # 6 m-chunks
BT = B // 128               # 2 batch tiles
BL = B // NCORES            # 32 batch rows per core in stage 2

_DT_NAME = os.environ.get("KRON_DTYPE", "bf16")
if _DT_NAME == "bf16":
    _DT_MM = mybir.dt.bfloat16
    _NP_MM = ml_dtypes.bfloat16
elif _DT_NAME == "fp32":
    _DT_MM = mybir.dt.float32
    _NP_MM = np.float32
else:
    raise ValueError(_DT_NAME)

_F32 = mybir.dt.float32


# ---------------------------------------------------------------- stage 1

def _build_stage1():
    nc = bacc.Bacc("TRN2", target_bir_lowering=False, debug=False,
                   num_devices=NCORES)
    w = nc.dram_tensor("w", [PLOC, 128, MCH, K], _DT_MM,
                       kind="ExternalInput").ap()
    mt = nc.dram_tensor("mt", [128, MCH, B], _DT_MM,
                        kind="ExternalInput").ap()
    ptb = nc.dram_tensor("ptb", [128, PLOC, B], _DT_MM,
                         kind="ExternalInput").ap()
    out = nc.dram_tensor("partial", [BT, 128, K], _F32,
                         kind="ExternalOutput").ap()

    with tile.TileContext(nc) as tc:
        with tc.tile_pool(name="single", bufs=1) as single, \
             tc.tile_pool(name="wp", bufs=6) as wp, \
             tc.tile_pool(name="scp", bufs=3) as scp, \
             tc.tile_pool(name="psp", bufs=1, space="PSUM") as psp, \
             tc.tile_pool(name="op", bufs=1) as op:

            mt_sb = single.tile([128, MCH, B], _DT_MM, tag="mt")
            nc.sync.dma_start(out=mt_sb, in_=mt)
            ptb_sb = single.tile([128, PLOC, B], _DT_MM, tag="ptb")
            nc.sync.dma_start(out=ptb_sb, in_=ptb)

            psums = [psp.tile([128, K], _F32, name=f"ps{i}", tag=f"ps{i}")
                     for i in range(BT)]

            for p in range(PLOC):
                wt = wp.tile([128, MCH, K], _DT_MM, tag="w")
                nc.sync.dma_start(out=wt, in_=w[p])

                sc = scp.tile([128, MCH, B], _DT_MM, tag="sc")
                in0, in1 = bass.broadcast_tensor_aps(
                    mt_sb[:, :, :], ptb_sb[:, p:p + 1, :])
                nc.vector.tensor_mul(out=sc, in0=in0, in1=in1)

                for mc in range(MCH):
                    for bt in range(BT):
                        nc.tensor.matmul(
                            psums[bt],
                            sc[:, mc, ds(bt * 128, 128)],
                            wt[:, mc, :],
                            start=(p == 0 and mc == 0),
                            stop=(p == PLOC - 1 and mc == MCH - 1),
                        )

            for bt in range(BT):
                ot = op.tile([128, K], _F32, tag=f"o{bt}")
                nc.vector.tensor_copy(out=ot, in_=psums[bt])
                nc.sync.dma_start(out=out[bt], in_=ot)
    nc.finalize()
    return nc


def _prep_stage1_inputs(x, W1):
    p = np.ascontiguousarray(x[:, :P_DIM])            # [B, P]
    m = np.ascontiguousarray(x[:, P_DIM:])            # [B, M]
    # mT [128, MCH, B]: mT_sb[mp, mc, b] = m[b, mc*128 + mp]
    mt = np.ascontiguousarray(
        m.T.reshape(MCH, 128, B).transpose(1, 0, 2)).astype(_NP_MM)
    # W: [K, P*M] -> [P, 128(m), MCH, K] per core slice along P
    W1r = W1.reshape(K, P_DIM, M_DIM)
    # -> [P, M, K]
    Wpmk = np.ascontiguousarray(W1r.transpose(1, 2, 0)).astype(_NP_MM)
    # -> [P, MCH, 128, K] -> [P, 128, MCH, K]
    Wfin = np.ascontiguousarray(
        Wpmk.reshape(P_DIM, MCH, 128, K).transpose(0, 2, 1, 3))
    pT = p.T.astype(_NP_MM)                           # [P, B]
    in_maps = []
    for c in range(NCORES):
        pt_c = pT[c * PLOC:(c + 1) * PLOC]            # [PLOC, B]
        ptb_c = np.ascontiguousarray(
            np.broadcast_to(pt_c[None, :, :], (128, PLOC, B)))
        in_maps.append({
            "w": np.ascontiguousarray(Wfin[c * PLOC:(c + 1) * PLOC]),
            "mt": mt,
            "ptb": ptb_c,
        })
    return in_maps


# ---------------------------------------------------------------- stage 2

def _build_stage2():
    nc = bacc.Bacc("TRN2", target_bir_lowering=False, debug=False,
                   num_devices=NCORES)
    parts = nc.dram_tensor("parts", [BL, NCORES, K], _F32,
                           kind="ExternalInput").ap()
    b1v = nc.dram_tensor("b1v", [K], _F32, kind="ExternalInput").ap()
    lng = nc.dram_tensor("lng", [K], _F32, kind="ExternalInput").ap()
    lnb = nc.dram_tensor("lnb", [K], _F32, kind="ExternalInput").ap()
    w2t = nc.dram_tensor("w2t", [128, K // 128, HID], _F32,
                         kind="ExternalInput").ap()
    s2 = nc.dram_tensor("s2", [HID], _F32, kind="ExternalInput").ap()
    c2 = nc.dram_tensor("c2", [HID], _F32, kind="ExternalInput").ap()
    w3t = nc.dram_tensor("w3t", [128, HID // 128, H2], _F32,
                         kind="ExternalInput").ap()
    s3 = nc.dram_tensor("s3", [H2], _F32, kind="ExternalInput").ap()
    c3 = nc.dram_tensor("c3", [H2], _F32, kind="ExternalInput").ap()
    w4t = nc.dram_tensor("w4t", [H2, NCLS], _F32, kind="ExternalInput").ap()
    b4v = nc.dram_tensor("b4v", [NCLS], _F32, kind="ExternalInput").ap()
    out = nc.dram_tensor("logits", [BL, NCLS], _F32,
                         kind="ExternalOutput").ap()

    def bcast(ap_1d, n):
        return bass.AP(tensor=ap_1d.tensor, offset=ap_1d.offset,
                       ap=[[0, BL]] + list(ap_1d.ap))

    with tile.TileContext(nc) as tc:
        with tc.tile_pool(name="single", bufs=1) as single, \
             tc.tile_pool(name="work", bufs=2) as work, \
             tc.tile_pool(name="psp", bufs=2, space="PSUM") as psp:

            ident = single.tile([BL, BL], _F32, tag="ident")
            make_identity(nc, ident)

            # broadcast 1-D vectors across the BL partitions
            vecs = {}
            for name, ap1, n in (("b1v", b1v, K), ("lng", lng, K),
                                 ("lnb", lnb, K), ("s2", s2, HID),
                                 ("c2", c2, HID), ("s3", s3, H2),
                                 ("c3", c3, H2), ("b4v", b4v, NCLS)):
                t = single.tile([BL, n], _F32, name="v_" + name, tag=name)
                nc.gpsimd.dma_start(out=t, in_=bcast(ap1, n))
                vecs[name] = t

            w2_sb = single.tile([128, K // 128, HID], _F32, tag="w2")
            nc.sync.dma_start(out=w2_sb, in_=w2t)
            w3_sb = single.tile([128, HID // 128, H2], _F32, tag="w3")
            nc.sync.dma_start(out=w3_sb, in_=w3t)
            w4_sb = single.tile([H2, NCLS], _F32, tag="w4")
            nc.sync.dma_start(out=w4_sb, in_=w4t)

            pin = work.tile([BL, NCORES, K], _F32, tag="pin")
            nc.sync.dma_start(out=pin, in_=parts)

            acc = work.tile([BL, K], _F32, tag="acc")
            nc.vector.tensor_add(out=acc, in0=pin[:, 0, :], in1=pin[:, 1, :])
            for c in range(2, NCORES):
                nc.vector.tensor_add(out=acc, in0=acc, in1=pin[:, c, :])
            nc.vector.tensor_add(out=acc, in0=acc, in1=vecs["b1v"])

            # LayerNorm over K (free dim)
            stats = work.tile([BL, 6], _F32, tag="stats")
            nc.vector.bn_stats(out=stats, in_=acc)
            mv = work.tile([BL, 2], _F32, tag="mv")
            nc.vector.bn_aggr(out=mv, in_=stats)
            epst = single.tile([BL, 1], _F32, tag="eps")
            nc.vector.memset(epst, EPS)
            rstd = work.tile([BL, 1], _F32, tag="rstd")
            nc.scalar.activation(out=rstd, in_=mv[:, 1:2],
                                 func=mybir.ActivationFunctionType.Sqrt,
                                 bias=epst, scale=1.0)
            nc.vector.reciprocal(out=rstd, in_=rstd)
            nc.vector.tensor_scalar(out=acc, in0=acc,
                                    scalar1=mv[:, 0:1], scalar2=rstd,
                                    op0=mybir.AluOpType.subtract,
                                    op1=mybir.AluOpType.mult)
            nc.vector.tensor_mul(out=acc, in0=acc, in1=vecs["lng"])
            nc.vector.tensor_add(out=acc, in0=acc, in1=vecs["lnb"])
            nc.vector.tensor_scalar_max(out=acc, in0=acc, scalar1=0.0)

            def dense(h_sb, nin, w_sb, nout):
                """out_psum[BL, nout] = h_sb[BL, nin] @ w_sb([128, nin/128, nout])"""
                nch = nin // 128
                ht = work.tile([128, nch, BL], _F32, tag=f"ht{nin}_{nout}")
                for kc in range(nch):
                    pst = psp.tile([128, BL], _F32, name="pst", tag="pst")
                    nc.tensor.transpose(pst, h_sb[:, ds(kc * 128, 128)], ident)
                    nc.vector.tensor_copy(out=ht[:, kc, :], in_=pst)
                ps = psp.tile([BL, nout], _F32, name=f"mm{nout}", tag="mm")
                for kc in range(nch):
                    nc.tensor.matmul(ps, ht[:, kc, :], w_sb[:, kc, :],
                                     start=(kc == 0), stop=(kc == nch - 1))
                return ps

            ps2 = dense(acc, K, w2_sb, HID)
            h2 = work.tile([BL, HID], _F32, tag="h2")
            nc.vector.tensor_mul(out=h2, in0=ps2, in1=vecs["s2"])
            nc.vector.tensor_add(out=h2, in0=h2, in1=vecs["c2"])
            nc.vector.tensor_scalar_max(out=h2, in0=h2, scalar1=0.0)

            ps3 = dense(h2, HID, w3_sb, H2)
            h3 = work.tile([BL, H2], _F32, tag="h3")
            nc.vector.tensor_mul(out=h3, in0=ps3, in1=vecs["s3"])
            nc.vector.tensor_add(out=h3, in0=h3, in1=vecs["c3"])
            nc.vector.tensor_scalar_max(out=h3, in0=h3, scalar1=0.0)

            # final: [BL, H2] @ [H2, NCLS]
            ht4 = work.tile([H2, BL], _F32, tag="ht4")
            pst4 = psp.tile([H2, BL], _F32, name="pst4", tag="pst")
            nc.tensor.transpose(pst4, h3, ident)
            nc.vector.tensor_copy(out=ht4, in_=pst4)
            ps4 = psp.tile([BL, NCLS], _F32, name="ps4", tag="mm")
            nc.tensor.matmul(ps4, ht4, w4_sb, start=True, stop=True)
            lg = work.tile([BL, NCLS], _F32, tag="lg")
            nc.vector.tensor_add(out=lg, in0=ps4, in1=vecs["b4v"])
            nc.sync.dma_start(out=out, in_=lg)
    nc.finalize()
    return nc


def _prep_stage2_inputs(partials, b1, ln_g, ln_b, W2, b2, bn1_g, bn1_b,
                        W3, b3, bn2_g, bn2_b, W4, b4):
    f32 = np.float32
    bnscale = f32(1.0 / np.sqrt(1.0 + EPS))
    s2 = (bn1_g * bnscale).astype(f32)
    c2 = (b2 * s2 + bn1_b).astype(f32)
    s3 = (bn2_g * bnscale).astype(f32)
    c3 = (b3 * s3 + bn2_b).astype(f32)
    w2t = np.ascontiguousarray(
        W2.T.reshape(K // 128, 128, HID).transpose(1, 0, 2)).astype(f32)
    w3t = np.ascontiguousarray(
        W3.T.reshape(HID // 128, 128, H2).transpose(1, 0, 2)).astype(f32)
    w4t = np.ascontiguousarray(W4.T).astype(f32)
    # partials: [NCORES, B, K] -> per stage-2 core c: [BL, NCORES, K]
    pstack = np.ascontiguousarray(
        np.stack(partials, axis=0).transpose(1, 0, 2))  # [B, NCORES, K]
    common = dict(b1v=b1.astype(f32), lng=ln_g.astype(f32),
                  lnb=ln_b.astype(f32), w2t=w2t, s2=s2, c2=c2,
                  w3t=w3t, s3=s3, c3=c3, w4t=w4t, b4v=b4.astype(f32))
    in_maps = []
    for c in range(NCORES):
        m = dict(common)
        m["parts"] = np.ascontiguousarray(pstack[c * BL:(c + 1) * BL])
        in_maps.append(m)
    return in_maps


# ---------------------------------------------------------------- driver

_CACHE = {}


def _get_nc(name, builder):
    if name not in _CACHE:
        _CACHE[name] = builder()
    return _CACHE[name]


def _run(nc, in_maps, trace, tag=""):
    kw = {}
    if trace:
        kw = dict(trace=True)
        td = os.environ.get("KRON_TRACE_DIR")
        if td:
            d = os.path.join(td, tag or "launch")
            os.makedirs(d, exist_ok=True)
            kw["tmpdir"] = d
    return run_bass_kernel_spmd(nc, in_maps, list(range(NCORES)), **kw)


def kernel_impl(inputs, trace=False):
    x = np.asarray(inputs["x"], np.float32)
    core_ids = list(range(NCORES))

    nc1 = _get_nc("s1", _build_stage1)
    in1 = _prep_stage1_inputs(x, np.asarray(inputs["W1"], np.float32))
    r1 = _run(nc1, in1, trace, "s1")
    partials = [r1.results[c]["partial"].reshape(B, K).astype(np.float32)
                for c in core_ids]

    nc2 = _get_nc("s2", _build_stage2)
    in2 = _prep_stage2_inputs(
        partials,
        *[np.asarray(inputs[k], np.float32) for k in
          ("b1", "ln_g", "ln_b", "W2", "b2", "bn1_g", "bn1_b",
           "W3", "b3", "bn2_g", "bn2_b", "W4", "b4")])
    r2 = _run(nc2, in2, trace, "s2")
    logits = np.concatenate(
        [r2.results[c]["logits"] for c in core_ids], axis=0)
    times = (r1.exec_time_ns, r2.exec_time_ns)
    return logits.astype(np.float32), times


def kernel(**inputs):
    logits, _ = kernel_impl(inputs)
    return logits
